# revision 29
# baseline (speedup 1.0000x reference)
"""Trainium2 SPMD kernel for nn_Attentionlayer_9208409883387.

Mathematical simplification: the reference computes
    h   = x @ W
    att = softmax(mask(leaky_relu(s1+s2), adj), axis=3)
    res = leaky_relu(h * sum_j att[..., j])
The row-sum of a softmax along its normalization axis is identically 1
(every row has >=1 unmasked entry: P[all-zero adj row] ~ 2^-1024), so
    res = leaky_relu(x @ W)
exactly, up to fp32 rounding of the softmax row-sum.

Strategy (v5, bf16): data-parallel over 48*1024 = 49152 rows, 6144
rows/core, packed host-side in bf16 with f_in on partitions
(xpack[0:64] = rows[0:3072].T, xpack[64:128] = rows[3072:].T) and W
replicated block-diagonally.  The first input transfer (W+c0+c1) is
split by partition halves across BOTH HWDGE rings — disjoint SDMA
engine sets move the halves concurrently, halving time-to-first-matmul.
Remaining input and all outputs use >=2KB-per-partition descriptors
(1KB descriptors measured ~2x slower).  Per 512-col chunk the PE runs
two 64x64 quadrant matmuls at tile positions (0,0)/(64,64) which
execute concurrently.  leaky_relu: ACT engine Lrelu over 2-bank PSUM
spans (banks 0-1, 2-3) + single bank 5; DVE covers bank 4 with
tmp=ps*0.01; max(ps,tmp).  Teardown (~7us walrus semaphore-reset
ladder + barriers) and preamble (~7us) are toolchain-fixed; the kernel
minimizes the last-output-receipt time which gates them.
"""

import numpy as np

B, T, N, F = 4, 12, 1024, 64
N_CORES = 8
ROWS = B * T * N              # 49152
RPC = ROWS // N_CORES         # 6144 rows per core
HALF = RPC // 2               # 3072 packed columns per core
CHUNK = 512                   # one PSUM bank of fp32 accumulators
NCHUNK = HALF // CHUNK        # 6

_PROGRAM = None


def _build_program():
    import concourse.bass as bass
    import concourse.mybir as mybir
    from contextlib import ExitStack

    bf16 = mybir.dt.bfloat16
    f32 = mybir.dt.float32
    nc = bass.Bass("TRN2")
    xp = nc.declare_dram_parameter("xpack", [128, 128 + HALF], bf16, isOutput=False)
    yp = nc.declare_dram_parameter("ypack", [128, HALF], bf16, isOutput=True)

    def xc(i):  # xpack column range of chunk i
        return 128 + i * CHUNK, 128 + (i + 1) * CHUNK

    with ExitStack() as ctx:
        x_sb = ctx.enter_context(nc.sbuf_tensor("x_sb", [128, 128 + HALF], bf16))
        y_sb = ctx.enter_context(nc.sbuf_tensor("y_sb", [128, HALF], bf16))
        tmpD = ctx.enter_context(nc.sbuf_tensor("tmpD", [128, CHUNK], bf16))
        # banks 0-1 and 2-3 as 2-bank tensors so one ACT Lrelu spans both
        ps01 = ctx.enter_context(nc.psum_tensor("ps01", [128, 2 * CHUNK], f32))
        ps23 = ctx.enter_context(nc.psum_tensor("ps23", [128, 2 * CHUNK], f32))
        ps4 = ctx.enter_context(nc.psum_tensor("ps4", [128, CHUNK], f32))
        ps5 = ctx.enter_context(nc.psum_tensor("ps5", [128, CHUNK], f32))
        psc = [
            ps01[:, 0:CHUNK], ps01[:, CHUNK : 2 * CHUNK],
            ps23[:, 0:CHUNK], ps23[:, CHUNK : 2 * CHUNK],
            ps4[:], ps5[:],
        ]
        sT = ctx.enter_context(nc.semaphore("sT"))    # in: W+c0+c1 top (SP)
        sBt = ctx.enter_context(nc.semaphore("sBt"))  # in: W+c0+c1 bot (ACT)
        sA2 = ctx.enter_context(nc.semaphore("sA2"))  # in: c4+c5 (SP ring)
        sG = ctx.enter_context(nc.semaphore("sG"))    # in: c2+c3 (SWDGE)
        pe_sem = ctx.enter_context(nc.semaphore("pe_sem"))  # +1 per chunk
        vA = ctx.enter_context(nc.semaphore("vA"))    # ACT lrelu spans done
        vD = ctx.enter_context(nc.semaphore("vD"))    # DVE lrelu bank4 done
        out_sem = ctx.enter_context(nc.semaphore("out_sem"))
        block = ctx.enter_context(nc.Block())

        @block.sync
        def _(sync):
            # in: W+c0+c1 top half — the bottom half flows concurrently on
            # the ACT ring via a disjoint SDMA engine set (2304B/partition)
            sync.dma_start(out=x_sb[0:64, 0:1152], in_=xp[0:64, 0:1152]).then_inc(
                sT, 16
            )
            sync.dma_start(out=x_sb[:, 2176:3200], in_=xp[:, 2176:3200]).then_inc(
                sA2, 16
            )
            sync.wait_ge(vA, 1)
            sync.dma_start(out=yp[:, 0:1024], in_=y_sb[:, 0:1024]).then_inc(
                out_sem, 16
            )
            sync.wait_ge(vA, 2)
            sync.dma_start(out=yp[:, 1024:2048], in_=y_sb[:, 1024:2048]).then_inc(
                out_sem, 16
            )
            # last out group split by partition halves across both rings
            sync.wait_ge(vD, 1)
            sync.wait_ge(vA, 3)
            sync.dma_start(
                out=yp[0:64, 2048:3072], in_=y_sb[0:64, 2048:3072]
            ).then_inc(out_sem, 16)
            # all 4 out DMAs (both rings) complete before teardown
            sync.wait_ge(out_sem, 64)

        @block.scalar
        def _(scalar):
            # in: W+c0+c1 bottom half on the ACT HWDGE ring
            scalar.dma_start(
                out=x_sb[64:128, 0:1152], in_=xp[64:128, 0:1152]
            ).then_inc(sBt, 16)
            # Touch the Lrelu table so the lazy ACT_TABLE_LOAD (~1.3us)
            # happens during the DMA in-stream.
            nc.scalar.activation(
                tmpD[0:1, 0:4], tmpD[0:1, 0:4],
                mybir.ActivationFunctionType.Lrelu, alpha=0.01,
            )
            scalar.wait_ge(pe_sem, 2)
            nc.scalar.activation(
                y_sb[:, 0:1024], ps01[:],
                mybir.ActivationFunctionType.Lrelu, alpha=0.01,
            ).then_inc(vA, 1)
            scalar.wait_ge(pe_sem, 4)
            nc.scalar.activation(
                y_sb[:, 1024:2048], ps23[:],
                mybir.ActivationFunctionType.Lrelu, alpha=0.01,
            ).then_inc(vA, 1)
            # bank 5 lrelu (GpSimd cannot access PSUM; DVE has bank 4)
            scalar.wait_ge(pe_sem, 6)
            nc.scalar.activation(
                y_sb[:, 2560:3072], ps5[:],
                mybir.ActivationFunctionType.Lrelu, alpha=0.01,
            ).then_inc(vA, 1)
            scalar.wait_ge(vD, 1)
            scalar.dma_start(
                out=yp[64:128, 2048:3072], in_=y_sb[64:128, 2048:3072]
            ).then_inc(out_sem, 16)

        @block.tensor
        def _(tensor):
            w0 = x_sb[0:64, 0:64]        # W in quadrant rows 0:64
            w1 = x_sb[64:128, 64:128]    # W copy in quadrant rows 64:128
            for ci in range(NCHUNK):
                if ci == 0:
                    tensor.wait_ge(sT, 16)
                    tensor.wait_ge(sBt, 16)
                elif ci == 2:
                    tensor.wait_ge(sG, 16)
                elif ci == 4:
                    tensor.wait_ge(sA2, 16)
                lo, hi = xc(ci)
                nc.tensor.matmul(
                    psc[ci][0:64, :], w0, x_sb[0:64, lo:hi],
                    start=True, stop=True, tile_position=(0, 0),
                )
                nc.tensor.matmul(
                    psc[ci][64:128, :], w1, x_sb[64:128, lo:hi],
                    start=True, stop=True, tile_position=(64, 64),
                ).then_inc(pe_sem, 1)

        @block.vector
        def _(vector):
            # bank 4: tmp = ps*0.01 ; y = max(ps, tmp)
            vector.wait_ge(pe_sem, 5)
            nc.vector.tensor_scalar_mul(tmpD[:], ps4[:], 0.01)
            nc.vector.tensor_tensor(
                y_sb[:, 2048:2560], ps4[:], tmpD[:], op=mybir.AluOpType.max
            ).then_inc(vD, 1)

        @block.gpsimd
        def _(gpsimd):
            # in: chunks 2,3 via the software DGE queue — third parallel
            # DMA path; the GpSimd engine is otherwise idle.
            gpsimd.dma_start(out=x_sb[:, 1152:2176], in_=xp[:, 1152:2176]).then_inc(
                sG, 16
            )

    nc.finalize()
    return nc


def _get_program():
    global _PROGRAM
    if _PROGRAM is None:
        _PROGRAM = _build_program()
    return _PROGRAM


def _make_in_maps(x, W):
    import ml_dtypes

    bf16 = ml_dtypes.bfloat16
    xr = np.ascontiguousarray(x, dtype=np.float32).reshape(N_CORES, RPC, F)
    wpack = np.zeros((128, 128), dtype=bf16)
    wb = np.asarray(W, dtype=np.float32).astype(bf16)
    wpack[0:64, 0:64] = wb
    wpack[64:128, 64:128] = wb
    in_maps = []
    for c in range(N_CORES):
        xpack = np.empty((128, 128 + HALF), dtype=bf16)
        xpack[:, 0:128] = wpack
        xpack[0:64, 128:] = xr[c, 0:HALF].T.astype(bf16)
        xpack[64:128, 128:] = xr[c, HALF:].T.astype(bf16)
        in_maps.append({"xpack": xpack})
    return in_maps


def run_spmd(x, W, **spmd_kwargs):
    """Run the Bass program on 8 cores; returns (y_full, BassKernelResults)."""
    from concourse.bass_utils import run_bass_kernel_spmd

    in_maps = _make_in_maps(x, W)
    res = run_bass_kernel_spmd(
        _get_program(), in_maps, list(range(N_CORES)), **spmd_kwargs
    )
    y = np.empty((N_CORES, RPC, F), np.float32)
    for c in range(N_CORES):
        ypack = np.asarray(res.results[c]["ypack"]).astype(np.float32)
        y[c, 0:HALF] = ypack[0:64].T
        y[c, HALF:] = ypack[64:128].T
    return y.reshape(B, T, N, F), res


def kernel(x, adj, W, a):
    # adj and a are mathematically dead (softmax row-sum == 1); see module doc.
    y, _ = run_spmd(np.asarray(x), np.asarray(W, dtype=np.float32))
    return y


# revision 34
# speedup vs baseline: 1.1761x; 1.1761x over previous
"""Trainium2 SPMD kernel for nn_Attentionlayer_9208409883387.

Mathematical simplification: the reference computes
    h   = x @ W
    att = softmax(mask(leaky_relu(s1+s2), adj), axis=3)
    res = leaky_relu(h * sum_j att[..., j])
The row-sum of a softmax along its normalization axis is identically 1
(every row has >=1 unmasked entry: P[all-zero adj row] ~ 2^-1024), so
    res = leaky_relu(x @ W)
exactly, up to fp32 rounding of the softmax row-sum.

Strategy (v5, bf16): data-parallel over 48*1024 = 49152 rows, 6144
rows/core, packed host-side in bf16 with f_in on partitions
(xpack[0:64] = rows[0:3072].T, xpack[64:128] = rows[3072:].T) and W
replicated block-diagonally.  The first input transfer (W+c0+c1) is
split by partition halves across BOTH HWDGE rings — disjoint SDMA
engine sets move the halves concurrently, halving time-to-first-matmul.
Remaining input and all outputs use >=2KB-per-partition descriptors
(1KB descriptors measured ~2x slower).  Per 512-col chunk the PE runs
two 64x64 quadrant matmuls at tile positions (0,0)/(64,64) which
execute concurrently.  leaky_relu: ACT engine Lrelu over 2-bank PSUM
spans (banks 0-1, 2-3) + single bank 5; DVE covers bank 4 with
tmp=ps*0.01; max(ps,tmp).  Teardown (~7us walrus semaphore-reset
ladder + barriers) and preamble (~7us) are toolchain-fixed; the kernel
minimizes the last-output-receipt time which gates them.
"""

import numpy as np

B, T, N, F = 4, 12, 1024, 64
N_CORES = 8
ROWS = B * T * N              # 49152
RPC = ROWS // N_CORES         # 6144 rows per core
HALF = RPC // 2               # 3072 packed columns per core
CHUNK = 512                   # one PSUM bank of fp32 accumulators
NCHUNK = HALF // CHUNK        # 6

_PROGRAM = None


def _build_program():
    import concourse.bass as bass
    import concourse.mybir as mybir
    from contextlib import ExitStack

    bf16 = mybir.dt.bfloat16
    f32 = mybir.dt.float32
    nc = bass.Bass("TRN2")
    xp = nc.declare_dram_parameter("xpack", [128, 128 + HALF], bf16, isOutput=False)
    yp = nc.declare_dram_parameter("ypack", [128, HALF], bf16, isOutput=True)

    def xc(i):  # xpack column range of chunk i
        return 128 + i * CHUNK, 128 + (i + 1) * CHUNK

    with ExitStack() as ctx:
        x_sb = ctx.enter_context(nc.sbuf_tensor("x_sb", [128, 128 + HALF], bf16))
        y_sb = ctx.enter_context(nc.sbuf_tensor("y_sb", [128, HALF], bf16))
        tmpD = ctx.enter_context(nc.sbuf_tensor("tmpD", [128, CHUNK], bf16))
        # banks 0-1 and 2-3 as 2-bank tensors so one ACT Lrelu spans both
        ps01 = ctx.enter_context(nc.psum_tensor("ps01", [128, 2 * CHUNK], f32))
        ps23 = ctx.enter_context(nc.psum_tensor("ps23", [128, 2 * CHUNK], f32))
        ps4 = ctx.enter_context(nc.psum_tensor("ps4", [128, CHUNK], f32))
        ps5 = ctx.enter_context(nc.psum_tensor("ps5", [128, CHUNK], f32))
        psc = [
            ps01[:, 0:CHUNK], ps01[:, CHUNK : 2 * CHUNK],
            ps23[:, 0:CHUNK], ps23[:, CHUNK : 2 * CHUNK],
            ps4[:], ps5[:],
        ]
        sT = ctx.enter_context(nc.semaphore("sT"))    # in: W+c0+c1 (SP ring)
        sB1 = ctx.enter_context(nc.semaphore("sB1"))  # in: c2+c3 (ACT ring)
        sG = ctx.enter_context(nc.semaphore("sG"))    # in: c4+c5 (SWDGE)
        pe_sem = ctx.enter_context(nc.semaphore("pe_sem"))  # +1 per chunk
        vA = ctx.enter_context(nc.semaphore("vA"))    # ACT lrelu spans done
        vD = ctx.enter_context(nc.semaphore("vD"))    # DVE lrelu bank4 done
        out_sem = ctx.enter_context(nc.semaphore("out_sem"))
        block = ctx.enter_context(nc.Block())

        @block.sync
        def _(sync):
            # in: W + chunks 0,1 -> 2304B/partition descriptors
            sync.dma_start(out=x_sb[:, 0:1152], in_=xp[:, 0:1152]).then_inc(sT, 16)
            sync.wait_ge(vA, 1)
            sync.dma_start(out=yp[:, 0:1024], in_=y_sb[:, 0:1024]).then_inc(
                out_sem, 16
            )
            sync.wait_ge(vA, 2)
            sync.dma_start(out=yp[:, 1024:2048], in_=y_sb[:, 1024:2048]).then_inc(
                out_sem, 16
            )
            # last out group split by partition halves across both rings
            sync.wait_ge(vD, 1)
            sync.wait_ge(vA, 3)
            sync.dma_start(
                out=yp[0:64, 2048:3072], in_=y_sb[0:64, 2048:3072]
            ).then_inc(out_sem, 16)
            # all 4 out DMAs (both rings) complete before teardown
            sync.wait_ge(out_sem, 64)

        @block.scalar
        def _(scalar):
            # in: chunks 2,3 on the ACT HWDGE ring (2048B/partition)
            scalar.dma_start(out=x_sb[:, 1152:2176], in_=xp[:, 1152:2176]).then_inc(
                sB1, 16
            )
            # Touch the Lrelu table so the lazy ACT_TABLE_LOAD (~1.3us)
            # happens during the DMA in-stream.
            nc.scalar.activation(
                tmpD[0:1, 0:4], tmpD[0:1, 0:4],
                mybir.ActivationFunctionType.Lrelu, alpha=0.01,
            )
            scalar.wait_ge(pe_sem, 2)
            nc.scalar.activation(
                y_sb[:, 0:1024], ps01[:],
                mybir.ActivationFunctionType.Lrelu, alpha=0.01,
            ).then_inc(vA, 1)
            scalar.wait_ge(pe_sem, 4)
            nc.scalar.activation(
                y_sb[:, 1024:2048], ps23[:],
                mybir.ActivationFunctionType.Lrelu, alpha=0.01,
            ).then_inc(vA, 1)
            # bank 5 lrelu (GpSimd cannot access PSUM; DVE has bank 4)
            scalar.wait_ge(pe_sem, 6)
            nc.scalar.activation(
                y_sb[:, 2560:3072], ps5[:],
                mybir.ActivationFunctionType.Lrelu, alpha=0.01,
            ).then_inc(vA, 1)
            scalar.wait_ge(vD, 1)
            scalar.dma_start(
                out=yp[64:128, 2048:3072], in_=y_sb[64:128, 2048:3072]
            ).then_inc(out_sem, 16)

        @block.tensor
        def _(tensor):
            w0 = x_sb[0:64, 0:64]        # W in quadrant rows 0:64
            w1 = x_sb[64:128, 64:128]    # W copy in quadrant rows 64:128
            waits = {0: sT, 2: sB1, 4: sG}
            for ci in range(NCHUNK):
                if ci in waits:
                    tensor.wait_ge(waits[ci], 16)
                lo, hi = xc(ci)
                nc.tensor.matmul(
                    psc[ci][0:64, :], w0, x_sb[0:64, lo:hi],
                    start=True, stop=True, tile_position=(0, 0),
                )
                nc.tensor.matmul(
                    psc[ci][64:128, :], w1, x_sb[64:128, lo:hi],
                    start=True, stop=True, tile_position=(64, 64),
                ).then_inc(pe_sem, 1)

        @block.vector
        def _(vector):
            # bank 4: tmp = ps*0.01 ; y = max(ps, tmp)
            vector.wait_ge(pe_sem, 5)
            nc.vector.tensor_scalar_mul(tmpD[:], ps4[:], 0.01)
            nc.vector.tensor_tensor(
                y_sb[:, 2048:2560], ps4[:], tmpD[:], op=mybir.AluOpType.max
            ).then_inc(vD, 1)

        @block.gpsimd
        def _(gpsimd):
            # in: chunks 4,5 via the software DGE queue — third parallel
            # DMA path; the GpSimd engine is otherwise idle.
            gpsimd.dma_start(out=x_sb[:, 2176:3200], in_=xp[:, 2176:3200]).then_inc(
                sG, 16
            )

    nc.finalize()
    return nc


def _get_program():
    global _PROGRAM
    if _PROGRAM is None:
        _PROGRAM = _build_program()
    return _PROGRAM


def _make_in_maps(x, W):
    import ml_dtypes

    bf16 = ml_dtypes.bfloat16
    xr = np.ascontiguousarray(x, dtype=np.float32).reshape(N_CORES, RPC, F)
    wpack = np.zeros((128, 128), dtype=bf16)
    wb = np.asarray(W, dtype=np.float32).astype(bf16)
    wpack[0:64, 0:64] = wb
    wpack[64:128, 64:128] = wb
    in_maps = []
    for c in range(N_CORES):
        xpack = np.empty((128, 128 + HALF), dtype=bf16)
        xpack[:, 0:128] = wpack
        xpack[0:64, 128:] = xr[c, 0:HALF].T.astype(bf16)
        xpack[64:128, 128:] = xr[c, HALF:].T.astype(bf16)
        in_maps.append({"xpack": xpack})
    return in_maps


def run_spmd(x, W, **spmd_kwargs):
    """Run the Bass program on 8 cores; returns (y_full, BassKernelResults)."""
    from concourse.bass_utils import run_bass_kernel_spmd

    in_maps = _make_in_maps(x, W)
    res = run_bass_kernel_spmd(
        _get_program(), in_maps, list(range(N_CORES)), **spmd_kwargs
    )
    y = np.empty((N_CORES, RPC, F), np.float32)
    for c in range(N_CORES):
        ypack = np.asarray(res.results[c]["ypack"]).astype(np.float32)
        y[c, 0:HALF] = ypack[0:64].T
        y[c, HALF:] = ypack[64:128].T
    return y.reshape(B, T, N, F), res


def kernel(x, adj, W, a):
    # adj and a are mathematically dead (softmax row-sum == 1); see module doc.
    y, _ = run_spmd(np.asarray(x), np.asarray(W, dtype=np.float32))
    return y


# revision 36
# speedup vs baseline: 1.1957x; 1.0167x over previous
"""Trainium2 SPMD kernel for nn_Attentionlayer_9208409883387.

Mathematical simplification: the reference computes
    h   = x @ W
    att = softmax(mask(leaky_relu(s1+s2), adj), axis=3)
    res = leaky_relu(h * sum_j att[..., j])
The row-sum of a softmax along its normalization axis is identically 1
(every row has >=1 unmasked entry: P[all-zero adj row] ~ 2^-1024), so
    res = leaky_relu(x @ W)
exactly, up to fp32 rounding of the softmax row-sum.

Strategy (v5, bf16): data-parallel over 48*1024 = 49152 rows, 6144
rows/core, packed host-side in bf16 with f_in on partitions
(xpack[0:64] = rows[0:3072].T, xpack[64:128] = rows[3072:].T) and W
replicated block-diagonally.  The first input transfer (W+c0+c1) is
split by partition halves across BOTH HWDGE rings — disjoint SDMA
engine sets move the halves concurrently, halving time-to-first-matmul.
Remaining input and all outputs use >=2KB-per-partition descriptors
(1KB descriptors measured ~2x slower).  Per 512-col chunk the PE runs
two 64x64 quadrant matmuls at tile positions (0,0)/(64,64) which
execute concurrently.  leaky_relu: ACT engine Lrelu over 2-bank PSUM
spans (banks 0-1, 2-3) + single bank 5; DVE covers bank 4 with
tmp=ps*0.01; max(ps,tmp).  Teardown (~7us walrus semaphore-reset
ladder + barriers) and preamble (~7us) are toolchain-fixed; the kernel
minimizes the last-output-receipt time which gates them.
"""

import numpy as np

B, T, N, F = 4, 12, 1024, 64
N_CORES = 8
ROWS = B * T * N              # 49152
RPC = ROWS // N_CORES         # 6144 rows per core
HALF = RPC // 2               # 3072 packed columns per core
CHUNK = 512                   # one PSUM bank of fp32 accumulators
NCHUNK = HALF // CHUNK        # 6

_PROGRAM = None


def _build_program():
    import concourse.bass as bass
    import concourse.mybir as mybir
    from contextlib import ExitStack

    bf16 = mybir.dt.bfloat16
    f32 = mybir.dt.float32
    nc = bass.Bass("TRN2")
    xp = nc.declare_dram_parameter("xpack", [128, 128 + HALF], bf16, isOutput=False)
    yp = nc.declare_dram_parameter("ypack", [128, HALF], bf16, isOutput=True)

    def xc(i):  # xpack column range of chunk i
        return 128 + i * CHUNK, 128 + (i + 1) * CHUNK

    with ExitStack() as ctx:
        x_sb = ctx.enter_context(nc.sbuf_tensor("x_sb", [128, 128 + HALF], bf16))
        y_sb = ctx.enter_context(nc.sbuf_tensor("y_sb", [128, HALF], bf16))
        tmpD = ctx.enter_context(nc.sbuf_tensor("tmpD", [128, CHUNK], bf16))
        # banks 0-1 and 2-3 as 2-bank tensors so one ACT Lrelu spans both
        ps01 = ctx.enter_context(nc.psum_tensor("ps01", [128, 2 * CHUNK], f32))
        ps23 = ctx.enter_context(nc.psum_tensor("ps23", [128, 2 * CHUNK], f32))
        ps4 = ctx.enter_context(nc.psum_tensor("ps4", [128, CHUNK], f32))
        ps5 = ctx.enter_context(nc.psum_tensor("ps5", [128, CHUNK], f32))
        psc = [
            ps01[:, 0:CHUNK], ps01[:, CHUNK : 2 * CHUNK],
            ps23[:, 0:CHUNK], ps23[:, CHUNK : 2 * CHUNK],
            ps4[:], ps5[:],
        ]
        sT = ctx.enter_context(nc.semaphore("sT"))    # in: W+c0+c1 (SP ring)
        sB1 = ctx.enter_context(nc.semaphore("sB1"))  # in: c2+c3 (ACT ring)
        sG = ctx.enter_context(nc.semaphore("sG"))    # in: c4+c5 (SWDGE)
        pe_sem = ctx.enter_context(nc.semaphore("pe_sem"))  # +1 per chunk
        vA = ctx.enter_context(nc.semaphore("vA"))    # ACT lrelu spans done
        vD = ctx.enter_context(nc.semaphore("vD"))    # DVE lrelu bank4 done
        out_sem = ctx.enter_context(nc.semaphore("out_sem"))
        block = ctx.enter_context(nc.Block())

        @block.sync
        def _(sync):
            # in: W + chunks 0,1 -> 2304B/partition descriptors
            sync.dma_start(out=x_sb[:, 0:1152], in_=xp[:, 0:1152]).then_inc(sT, 16)
            sync.wait_ge(vA, 1)
            sync.dma_start(out=yp[:, 0:1024], in_=y_sb[:, 0:1024]).then_inc(
                out_sem, 16
            )
            sync.wait_ge(vA, 2)
            sync.dma_start(out=yp[:, 1024:2048], in_=y_sb[:, 1024:2048]).then_inc(
                out_sem, 16
            )
            # all 3 out DMAs (both rings) complete before teardown
            sync.wait_ge(out_sem, 48)

        @block.scalar
        def _(scalar):
            # in: chunks 2,3 on the ACT HWDGE ring (2048B/partition)
            scalar.dma_start(out=x_sb[:, 1152:2176], in_=xp[:, 1152:2176]).then_inc(
                sB1, 16
            )
            # Touch the Lrelu table so the lazy ACT_TABLE_LOAD (~1.3us)
            # happens during the DMA in-stream.
            nc.scalar.activation(
                tmpD[0:1, 0:4], tmpD[0:1, 0:4],
                mybir.ActivationFunctionType.Lrelu, alpha=0.01,
            )
            scalar.wait_ge(pe_sem, 2)
            nc.scalar.activation(
                y_sb[:, 0:1024], ps01[:],
                mybir.ActivationFunctionType.Lrelu, alpha=0.01,
            ).then_inc(vA, 1)
            scalar.wait_ge(pe_sem, 4)
            nc.scalar.activation(
                y_sb[:, 1024:2048], ps23[:],
                mybir.ActivationFunctionType.Lrelu, alpha=0.01,
            ).then_inc(vA, 1)
            # bank 5 lrelu (GpSimd cannot access PSUM; DVE has bank 4)
            scalar.wait_ge(pe_sem, 6)
            nc.scalar.activation(
                y_sb[:, 2560:3072], ps5[:],
                mybir.ActivationFunctionType.Lrelu, alpha=0.01,
            )
            scalar.wait_ge(vD, 1)
            scalar.dma_start(out=yp[:, 2048:3072], in_=y_sb[:, 2048:3072]).then_inc(
                out_sem, 16
            )

        @block.tensor
        def _(tensor):
            w0 = x_sb[0:64, 0:64]        # W in quadrant rows 0:64
            w1 = x_sb[64:128, 64:128]    # W copy in quadrant rows 64:128
            waits = {0: sT, 2: sB1, 4: sG}
            for ci in range(NCHUNK):
                if ci in waits:
                    tensor.wait_ge(waits[ci], 16)
                lo, hi = xc(ci)
                nc.tensor.matmul(
                    psc[ci][0:64, :], w0, x_sb[0:64, lo:hi],
                    start=True, stop=True, tile_position=(0, 0),
                )
                nc.tensor.matmul(
                    psc[ci][64:128, :], w1, x_sb[64:128, lo:hi],
                    start=True, stop=True, tile_position=(64, 64),
                ).then_inc(pe_sem, 1)

        @block.vector
        def _(vector):
            # bank 4: tmp = ps*0.01 ; y = max(ps, tmp)
            vector.wait_ge(pe_sem, 5)
            nc.vector.tensor_scalar_mul(tmpD[:], ps4[:], 0.01)
            nc.vector.tensor_tensor(
                y_sb[:, 2048:2560], ps4[:], tmpD[:], op=mybir.AluOpType.max
            ).then_inc(vD, 1)

        @block.gpsimd
        def _(gpsimd):
            # in: chunks 4,5 via the software DGE queue — third parallel
            # DMA path; the GpSimd engine is otherwise idle.
            gpsimd.dma_start(out=x_sb[:, 2176:3200], in_=xp[:, 2176:3200]).then_inc(
                sG, 16
            )

    nc.finalize()
    return nc


def _get_program():
    global _PROGRAM
    if _PROGRAM is None:
        _PROGRAM = _build_program()
    return _PROGRAM


def _make_in_maps(x, W):
    import ml_dtypes

    bf16 = ml_dtypes.bfloat16
    xr = np.ascontiguousarray(x, dtype=np.float32).reshape(N_CORES, RPC, F)
    wpack = np.zeros((128, 128), dtype=bf16)
    wb = np.asarray(W, dtype=np.float32).astype(bf16)
    wpack[0:64, 0:64] = wb
    wpack[64:128, 64:128] = wb
    in_maps = []
    for c in range(N_CORES):
        xpack = np.empty((128, 128 + HALF), dtype=bf16)
        xpack[:, 0:128] = wpack
        xpack[0:64, 128:] = xr[c, 0:HALF].T.astype(bf16)
        xpack[64:128, 128:] = xr[c, HALF:].T.astype(bf16)
        in_maps.append({"xpack": xpack})
    return in_maps


def run_spmd(x, W, **spmd_kwargs):
    """Run the Bass program on 8 cores; returns (y_full, BassKernelResults)."""
    from concourse.bass_utils import run_bass_kernel_spmd

    in_maps = _make_in_maps(x, W)
    res = run_bass_kernel_spmd(
        _get_program(), in_maps, list(range(N_CORES)), **spmd_kwargs
    )
    y = np.empty((N_CORES, RPC, F), np.float32)
    for c in range(N_CORES):
        ypack = np.asarray(res.results[c]["ypack"]).astype(np.float32)
        y[c, 0:HALF] = ypack[0:64].T
        y[c, HALF:] = ypack[64:128].T
    return y.reshape(B, T, N, F), res


def kernel(x, adj, W, a):
    # adj and a are mathematically dead (softmax row-sum == 1); see module doc.
    y, _ = run_spmd(np.asarray(x), np.asarray(W, dtype=np.float32))
    return y


# revision 37
# speedup vs baseline: 1.2101x; 1.0120x over previous
"""Trainium2 SPMD kernel for nn_Attentionlayer_9208409883387.

Mathematical simplification: the reference computes
    h   = x @ W
    att = softmax(mask(leaky_relu(s1+s2), adj), axis=3)
    res = leaky_relu(h * sum_j att[..., j])
The row-sum of a softmax along its normalization axis is identically 1
(every row has >=1 unmasked entry: P[all-zero adj row] ~ 2^-1024), so
    res = leaky_relu(x @ W)
exactly, up to fp32 rounding of the softmax row-sum.

Strategy (bf16, 3 DMA paths): data-parallel over 48*1024 = 49152 rows,
6144 rows/core, packed host-side in bf16 with f_in on partitions
(xpack[0:64] = rows[0:3072].T, xpack[64:128] = rows[3072:].T) and W
replicated block-diagonally.  Inputs stream on three concurrent DMA
paths — SP HWDGE ring (W+c0+c1), ACT HWDGE ring (c2+c3), and the
GpSimd SWDGE queue (c4+c5) — all with >=2KB-per-partition descriptors
(1KB descriptors measured ~2x slower).  Per 512-col chunk the PE runs
two 64x64 quadrant matmuls at tile positions (0,0)/(64,64) which
execute concurrently (~427ns/chunk vs ~630 for one block-diagonal
matmul).  leaky_relu: ACT engine Lrelu over 2-bank PSUM spans (banks
0-1, 2-3) + single bank 5; DVE covers bank 4 with tmp=ps*0.01;
max(ps,tmp) (a single instruction may read PSUM only once).  Outputs:
y0..y3 on the SP ring, y4+y5 on the ACT ring.  Measured ~25-26.5us
worst-core (vs 28.9us fp32 baseline); of that, ~7.2us NEFF preamble,
~7.4us walrus teardown (256-semaphore reset ladder behind an
all-engine barrier), and ~2.8us first-DMA latency are toolchain/HW
fixed.  Host upcasts the bf16 output to fp32 (rel L2 err ~2.9e-3,
gate 2e-2).
"""

import numpy as np

B, T, N, F = 4, 12, 1024, 64
N_CORES = 8
ROWS = B * T * N              # 49152
RPC = ROWS // N_CORES         # 6144 rows per core
HALF = RPC // 2               # 3072 packed columns per core
CHUNK = 512                   # one PSUM bank of fp32 accumulators
NCHUNK = HALF // CHUNK        # 6

_PROGRAM = None


def _build_program():
    import concourse.bass as bass
    import concourse.mybir as mybir
    from contextlib import ExitStack

    bf16 = mybir.dt.bfloat16
    f32 = mybir.dt.float32
    nc = bass.Bass("TRN2")
    xp = nc.declare_dram_parameter("xpack", [128, 128 + HALF], bf16, isOutput=False)
    yp = nc.declare_dram_parameter("ypack", [128, HALF], bf16, isOutput=True)

    def xc(i):  # xpack column range of chunk i
        return 128 + i * CHUNK, 128 + (i + 1) * CHUNK

    with ExitStack() as ctx:
        x_sb = ctx.enter_context(nc.sbuf_tensor("x_sb", [128, 128 + HALF], bf16))
        y_sb = ctx.enter_context(nc.sbuf_tensor("y_sb", [128, HALF], bf16))
        tmpD = ctx.enter_context(nc.sbuf_tensor("tmpD", [128, CHUNK], bf16))
        # banks 0-1 and 2-3 as 2-bank tensors so one ACT Lrelu spans both
        ps01 = ctx.enter_context(nc.psum_tensor("ps01", [128, 2 * CHUNK], f32))
        ps23 = ctx.enter_context(nc.psum_tensor("ps23", [128, 2 * CHUNK], f32))
        ps4 = ctx.enter_context(nc.psum_tensor("ps4", [128, CHUNK], f32))
        ps5 = ctx.enter_context(nc.psum_tensor("ps5", [128, CHUNK], f32))
        psc = [
            ps01[:, 0:CHUNK], ps01[:, CHUNK : 2 * CHUNK],
            ps23[:, 0:CHUNK], ps23[:, CHUNK : 2 * CHUNK],
            ps4[:], ps5[:],
        ]
        sT = ctx.enter_context(nc.semaphore("sT"))    # in: W+c0+c1 (SP ring)
        sB1 = ctx.enter_context(nc.semaphore("sB1"))  # in: c2+c3 (ACT ring)
        sG = ctx.enter_context(nc.semaphore("sG"))    # in: c4+c5 (SWDGE)
        pe_sem = ctx.enter_context(nc.semaphore("pe_sem"))  # +1 per chunk
        vA = ctx.enter_context(nc.semaphore("vA"))    # ACT lrelu spans done
        vD = ctx.enter_context(nc.semaphore("vD"))    # DVE lrelu bank4 done
        out_sem = ctx.enter_context(nc.semaphore("out_sem"))
        block = ctx.enter_context(nc.Block())

        @block.sync
        def _(sync):
            # in: W + chunks 0,1 -> 2304B/partition descriptors
            sync.dma_start(out=x_sb[:, 0:1152], in_=xp[:, 0:1152]).then_inc(sT, 16)
            sync.wait_ge(vA, 1)
            sync.dma_start(out=yp[:, 0:1024], in_=y_sb[:, 0:1024]).then_inc(
                out_sem, 16
            )
            sync.wait_ge(vA, 2)
            sync.dma_start(out=yp[:, 1024:2048], in_=y_sb[:, 1024:2048]).then_inc(
                out_sem, 16
            )
            # all 3 out DMAs (both rings) complete before teardown
            sync.wait_ge(out_sem, 48)

        @block.scalar
        def _(scalar):
            # in: chunks 2,3 on the ACT HWDGE ring (2048B/partition)
            scalar.dma_start(out=x_sb[:, 1152:2176], in_=xp[:, 1152:2176]).then_inc(
                sB1, 16
            )
            # Touch the Lrelu table so the lazy ACT_TABLE_LOAD (~1.3us)
            # happens during the DMA in-stream.
            nc.scalar.activation(
                tmpD[0:1, 0:4], tmpD[0:1, 0:4],
                mybir.ActivationFunctionType.Lrelu, alpha=0.01,
            )
            scalar.wait_ge(pe_sem, 2)
            nc.scalar.activation(
                y_sb[:, 0:1024], ps01[:],
                mybir.ActivationFunctionType.Lrelu, alpha=0.01,
            ).then_inc(vA, 1)
            scalar.wait_ge(pe_sem, 4)
            nc.scalar.activation(
                y_sb[:, 1024:2048], ps23[:],
                mybir.ActivationFunctionType.Lrelu, alpha=0.01,
            ).then_inc(vA, 1)
            # bank 5 lrelu (GpSimd cannot access PSUM; DVE has bank 4)
            scalar.wait_ge(pe_sem, 6)
            nc.scalar.activation(
                y_sb[:, 2560:3072], ps5[:],
                mybir.ActivationFunctionType.Lrelu, alpha=0.01,
            )
            scalar.wait_ge(vD, 1)
            scalar.dma_start(out=yp[:, 2048:3072], in_=y_sb[:, 2048:3072]).then_inc(
                out_sem, 16
            )

        @block.tensor
        def _(tensor):
            w0 = x_sb[0:64, 0:64]        # W in quadrant rows 0:64
            w1 = x_sb[64:128, 64:128]    # W copy in quadrant rows 64:128
            waits = {0: sT, 2: sB1, 4: sG}
            for ci in range(NCHUNK):
                if ci in waits:
                    tensor.wait_ge(waits[ci], 16)
                lo, hi = xc(ci)
                nc.tensor.matmul(
                    psc[ci][0:64, :], w0, x_sb[0:64, lo:hi],
                    start=True, stop=True, tile_position=(0, 0),
                )
                nc.tensor.matmul(
                    psc[ci][64:128, :], w1, x_sb[64:128, lo:hi],
                    start=True, stop=True, tile_position=(64, 64),
                ).then_inc(pe_sem, 1)

        @block.vector
        def _(vector):
            # bank 4: tmp = ps*0.01 ; y = max(ps, tmp)
            vector.wait_ge(pe_sem, 5)
            nc.vector.tensor_scalar_mul(tmpD[:], ps4[:], 0.01)
            nc.vector.tensor_tensor(
                y_sb[:, 2048:2560], ps4[:], tmpD[:], op=mybir.AluOpType.max
            ).then_inc(vD, 1)

        @block.gpsimd
        def _(gpsimd):
            # in: chunks 4,5 via the software DGE queue — third parallel
            # DMA path; the GpSimd engine is otherwise idle.
            gpsimd.dma_start(out=x_sb[:, 2176:3200], in_=xp[:, 2176:3200]).then_inc(
                sG, 16
            )

    nc.finalize()
    return nc


def _get_program():
    global _PROGRAM
    if _PROGRAM is None:
        _PROGRAM = _build_program()
    return _PROGRAM


def _make_in_maps(x, W):
    import ml_dtypes

    bf16 = ml_dtypes.bfloat16
    xr = np.ascontiguousarray(x, dtype=np.float32).reshape(N_CORES, RPC, F)
    wpack = np.zeros((128, 128), dtype=bf16)
    wb = np.asarray(W, dtype=np.float32).astype(bf16)
    wpack[0:64, 0:64] = wb
    wpack[64:128, 64:128] = wb
    in_maps = []
    for c in range(N_CORES):
        xpack = np.empty((128, 128 + HALF), dtype=bf16)
        xpack[:, 0:128] = wpack
        xpack[0:64, 128:] = xr[c, 0:HALF].T.astype(bf16)
        xpack[64:128, 128:] = xr[c, HALF:].T.astype(bf16)
        in_maps.append({"xpack": xpack})
    return in_maps


def run_spmd(x, W, **spmd_kwargs):
    """Run the Bass program on 8 cores; returns (y_full, BassKernelResults)."""
    from concourse.bass_utils import run_bass_kernel_spmd

    in_maps = _make_in_maps(x, W)
    res = run_bass_kernel_spmd(
        _get_program(), in_maps, list(range(N_CORES)), **spmd_kwargs
    )
    y = np.empty((N_CORES, RPC, F), np.float32)
    for c in range(N_CORES):
        ypack = np.asarray(res.results[c]["ypack"]).astype(np.float32)
        y[c, 0:HALF] = ypack[0:64].T
        y[c, HALF:] = ypack[64:128].T
    return y.reshape(B, T, N, F), res


def kernel(x, adj, W, a):
    # adj and a are mathematically dead (softmax row-sum == 1); see module doc.
    y, _ = run_spmd(np.asarray(x), np.asarray(W, dtype=np.float32))
    return y
